# revision 2
# baseline (speedup 1.0000x reference)
"""Trainium2 Bass kernel for a 4-layer binary MLP (BinaryFCNN).

Reference computation (per layer):  h = sign_pm1(h @ sign_pm1(W).T + b)
with x: [8192, 4096] fp32, W_l: [4096, 4096] fp32, b_l: [4096] fp32.

Strategy (v2)
-------------
* Data-parallel over the batch: core c processes rows [c*1024, (c+1)*1024).
  No collectives; every core streams all four weight matrices.
* All marshaling that doesn't depend on the matmul results happens on host
  (it is pure re-encoding of the inputs): x is split into fp16 digits
  hi = fp16(x), lo = fp16(x - hi) (the true residual; the PE honors fp16
  subnormals), and each W is pre-encoded as sign fp8 weights
  (w >= 0) - 0.5 in {-0.5, +0.5}.  This removes every DVE/ACT prep op from
  the device loop: the kernel is pure DMA + PE matmul + ACT sign.
* Layer 1 accumulates hi and lo passes into one PSUM tile, sharing the same
  fp8 +-0.5 stationary weights (products are exact exponent shifts, so the
  result carries ~22 significant bits of x; measured a single borderline
  sign flip vs the fp64 oracle).
* Layers 2..4 are bit-exact: +-1 activations and +-0.5 weights in fp8e4m3
  with fp32 PSUM accumulation; fp8 DoubleRow pairs k-chunks for
  2 fp8 MACs/PE/cycle.
* The sign activation uses ACT Sign(2*psum + b) == sign(h @ sign(W).T + b)
  with the per-feature bias as the ACT per-partition bias operand.
* A short burst of throwaway matmuls covers the first weight/x DMAs and
  releases the PE HAM clock gate before the real stream.

Per-core floor: layer 1 = 2 fp16-rate passes ~874 us, layers 2-4 = 3
DoubleRow passes ~655 us.
"""
import numpy as np
import ml_dtypes

import concourse.tile as tile
from concourse import bacc
import concourse.mybir as mybir
from concourse.bass_utils import run_bass_kernel_spmd

F32 = mybir.dt.float32
F16 = mybir.dt.float16
BF16 = mybir.dt.bfloat16
FP8 = mybir.dt.float8e4
SIGN = mybir.ActivationFunctionType.Sign

N_CORES = 8
D_FULL = 4096
B_FULL = 8192
MF = 512  # matmul moving free dim == one fp32 PSUM bank

# Probe-dependent switches (validated on hardware before shipping):
#   MIXED_W8:     fp8e4 +-0.5 stationary with fp16 moving operand works.
#   SUBNORMAL_LO: PE honors fp16 subnormal moving values (lo = raw residual).
MIXED_W8 = True
SUBNORMAL_LO = True


def build_binary_mlp(D: int, M: int, n_layers: int = 4) -> "bacc.Bacc":
    """Emit the per-core kernel. D = feature dim, M = per-core batch rows."""
    KO = D // 128  # contraction chunks (also input-feature blocks)
    NB = D // 128  # output-feature blocks
    MH = M // MF   # batch slices of the moving operand

    nc = bacc.Bacc("TRN2", target_bir_lowering=False, debug=False)
    l1_wdt = FP8 if MIXED_W8 else F16
    xhi = nc.declare_dram_parameter("xhi", [128, KO, M], F16, isOutput=False)
    xlo = nc.declare_dram_parameter("xlo", [128, KO, M], F16, isOutput=False)
    ws = [
        nc.declare_dram_parameter(
            f"w{l + 1}", [NB, 128, KO, 128], FP8 if l > 0 else l1_wdt, isOutput=False
        )
        for l in range(n_layers)
    ]
    bs = [
        nc.declare_dram_parameter(f"b{l + 1}", [128, NB], F32, isOutput=False)
        for l in range(n_layers)
    ]
    out = nc.declare_dram_parameter("out", [NB, 128, M], BF16, isOutput=True)

    with tile.TileContext(nc) as tc:
        with (
            tc.tile_pool(name="const", bufs=1) as const,
            tc.tile_pool(name="wpool", bufs=4) as wpool,
            tc.tile_pool(name="xio", bufs=4) as xio,
            tc.tile_pool(name="psum", bufs=5, space="PSUM") as psum,
            tc.tile_pool(name="psum1", bufs=1, space="PSUM") as psum1,
        ):
            bias_tiles = []
            for l in range(n_layers):
                bt = const.tile([128, NB], F32, tag=f"bias{l}", name=f"bias{l}")
                nc.sync.dma_start(bt[:], bs[l][:])
                bias_tiles.append(bt)

            # x digits, full per-core batch, chunked DMA so the first matmuls
            # only wait on their own chunks
            hi = const.tile([128, KO, M], F16, tag="hi", name="hi")
            lo = const.tile([128, KO, M], F16, tag="lo", name="lo")
            for ko in range(KO):
                nc.sync.dma_start(hi[:, ko, :], xhi[:, ko, :])
                nc.sync.dma_start(lo[:, ko, :], xlo[:, ko, :])

            # PE warm-up: covers the first weight/x DMAs and releases the HAM
            # clock gate (cold 1.2 GHz -> warm 2.4 GHz after ~3.4us).
            wu = const.tile([128, MF], F16, tag="warm", name="warm")
            nc.vector.memset(wu[:], 1.0)
            wps = psum1.tile([128, MF], F32, tag="wps", name="wps")
            n_wu = 40
            for i in range(n_wu):
                nc.tensor.matmul(wps[:], wu[:, :128], wu[:],
                                 start=(i == 0), stop=(i == n_wu - 1))

            # ping-pong activation buffers, feature-major, +-1 in fp8
            hA = const.tile([128, KO, M], FP8, tag="hA", name="hA")
            hB = const.tile([128, KO, M], FP8, tag="hB", name="hB")

            # ---------------- layer 1: fp16 hi/lo digit passes ----------------
            for nb in range(NB):
                wt = wpool.tile([128, KO, 128], l1_wdt, tag="w", name="wt")
                nc.sync.dma_start(wt[:], ws[0][nb])
                for mh in range(MH):
                    ms = slice(mh * MF, (mh + 1) * MF)
                    ps = psum.tile([128, MF], F32, tag="ps", name="ps")
                    for ko in range(KO):
                        nc.tensor.matmul(ps[:], wt[:, ko, :], hi[:, ko, ms],
                                         start=(ko == 0), stop=False)
                        nc.tensor.matmul(ps[:], wt[:, ko, :], lo[:, ko, ms],
                                         start=False, stop=(ko == KO - 1))
                    # h1 = Sign(2*psum + b) in {-1, +1} (psum = 0.5 * x@sign(W).T)
                    nc.scalar.activation(hA[:, nb, ms], ps[:], SIGN,
                                         bias=bias_tiles[0][:, nb:nb + 1], scale=2.0)

            # ---------------- layers 2..n: exact +-1 x +-0.5 ----------------
            hin, hout = hA, hB
            for l in range(1, n_layers):
                last = l == n_layers - 1
                for nb in range(NB):
                    wt = wpool.tile([128, KO, 128], FP8, tag="w", name="wt")
                    nc.sync.dma_start(wt[:], ws[l][nb])
                    for mh in range(MH):
                        ms = slice(mh * MF, (mh + 1) * MF)
                        ps = psum.tile([128, MF], F32, tag="ps", name="ps")
                        for ko in range(0, KO, 2):
                            nc.tensor.matmul(
                                ps[:], wt[:, ko:ko + 2, :], hin[:, ko:ko + 2, ms],
                                start=(ko == 0), stop=(ko + 2 == KO),
                                perf_mode=mybir.MatmulPerfMode.DoubleRow)
                        if last:
                            ot = xio.tile([128, MF], BF16, tag="ot", name="ot")
                            nc.scalar.activation(ot[:], ps[:], SIGN,
                                                 bias=bias_tiles[l][:, nb:nb + 1], scale=2.0)
                            nc.sync.dma_start(out[nb, :, ms], ot[:])
                        else:
                            nc.scalar.activation(hout[:, nb, ms], ps[:], SIGN,
                                                 bias=bias_tiles[l][:, nb:nb + 1], scale=2.0)
                hin, hout = hout, hin
    nc.compile()
    return nc


def _pack_w(W: np.ndarray, np_dt) -> np.ndarray:
    """W [D, D] fp32 -> [NB, 128(p=k_in), KO, 128(n)] sign weights in {-0.5, 0.5}
    with WP[nb, p, ko, n] = 0.5*sign_pm1(W[nb*128 + n, ko*128 + p])."""
    D = W.shape[0]
    nb = D // 128
    sw = np.where(W >= 0, 0.5, -0.5).astype(np_dt)
    return np.ascontiguousarray(sw.reshape(nb, 128, nb, 128).transpose(0, 3, 2, 1))


def _pack_b(b: np.ndarray) -> np.ndarray:
    return np.ascontiguousarray(b.astype(np.float32).reshape(-1, 128).T)


def _pack_x(xt: np.ndarray, KO: int) -> np.ndarray:
    """xt [D, M] -> [128, KO, M] with out[p, ko, m] = xt[ko*128 + p, m]."""
    D, M = xt.shape
    return np.ascontiguousarray(xt.reshape(KO, 128, M).transpose(1, 0, 2))


last_result = None  # BassKernelResults of the most recent run (for test.py)
_nc_cache = {}


def kernel(x, W1, b1, W2, b2, W3, b3, W4, b4):
    global last_result
    assert x.shape == (B_FULL, D_FULL)
    M = B_FULL // N_CORES
    KO = D_FULL // 128

    if (D_FULL, M) not in _nc_cache:
        _nc_cache[(D_FULL, M)] = build_binary_mlp(D_FULL, M)
    nc = _nc_cache[(D_FULL, M)]

    # host marshaling: pure re-encoding of the inputs
    xt = np.asarray(x, np.float32).T          # [D, B]
    xt_hi = xt.astype(np.float16)             # 11-bit digit
    xt_lo = (xt - xt_hi.astype(np.float32)).astype(np.float16)  # residual digit

    shared = {}
    l1_np_dt = ml_dtypes.float8_e4m3 if MIXED_W8 else np.float16
    for l, (W, b) in enumerate(((W1, b1), (W2, b2), (W3, b3), (W4, b4)), start=1):
        np_dt = ml_dtypes.float8_e4m3 if l > 1 else l1_np_dt
        shared[f"w{l}"] = _pack_w(np.asarray(W), np_dt)
        shared[f"b{l}"] = _pack_b(np.asarray(b))

    in_maps = []
    for c in range(N_CORES):
        m = dict(shared)
        m["xhi"] = _pack_x(xt_hi[:, c * M:(c + 1) * M], KO)
        m["xlo"] = _pack_x(xt_lo[:, c * M:(c + 1) * M], KO)
        in_maps.append(m)

    try:
        res = run_bass_kernel_spmd(nc, in_maps, core_ids=list(range(N_CORES)))
    except Exception:
        # one retry for transient device hiccups (NRT_EXEC_UNIT_UNRECOVERABLE
        # has been observed on otherwise healthy workers)
        res = run_bass_kernel_spmd(nc, in_maps, core_ids=list(range(N_CORES)))
    last_result = res

    parts = []
    for c in range(N_CORES):
        o = np.asarray(res.results[c]["out"])  # [NB, 128, M] bf16, values +-1
        parts.append(o.reshape(D_FULL, M).T)   # -> [M, D] (rows are batch)
    return np.concatenate(parts, axis=0).astype(np.float32)
